# revision 1
# baseline (speedup 1.0000x reference)
"""Llama4 MoE experts (grouped GEMM + SwiGLU) on 8 Trainium2 NeuronCores.

Expert-parallel: core e computes expert e's token block
  Y_e = (silu(X_e @ Wg_e) * (X_e @ Wu_e)) @ Wd_e
with X_e = hidden_states[e*1024:(e+1)*1024]. No collectives needed.

All matmuls run on the PE in fp32r (full-rate fp32 mode, inputs rounded
on-chip by DVE casts). Per-core dataflow (transposed activations):
  1. PE-transpose X into Xt (H on partitions, tokens free), fp32r.
  2. MM1: gup^T = Wgu_chunk.T @ Xt accumulated over H in PSUM;
     SwiGLU (sigmoid on ScalarE + two DVE muls) -> act^T fp32r.
  3. MM2: Y = act^T_slice.T @ Wd_chunk accumulated over D in PSUM;
     eviction on ScalarE, DMA out.
Tokens go in two halves of 512 so the Xt/act slabs fit in SBUF.
"""
from contextlib import ExitStack

import numpy as np

import concourse.bass as bass
import concourse.tile as tile
from concourse import bacc, mybir
from concourse.bass_utils import run_bass_kernel_spmd
from concourse.masks import make_identity

P = 128
F32 = mybir.dt.float32
F32R = mybir.dt.float32r
SIGMOID = mybir.ActivationFunctionType.Sigmoid
COPY = mybir.ActivationFunctionType.Copy

E = 8            # experts == cores
T = 1024         # tokens per expert
H = 4096         # hidden
D = 4096         # expert (intermediate) dim

_cached_nc = None


def _build_program(T=T, H=H, D=D, TH=512, MG=4, NW=512, w_bufs=6):
    halves = T // TH
    KH = H // P
    KD = D // P
    TT = TH // P
    GG = D // (MG * P)
    NH = H // NW
    XC = min(H, 1024)
    NXC = H // XC

    nc = bacc.Bacc("TRN2", target_bir_lowering=False, debug=False)
    x_d = nc.dram_tensor("x", [T, H], F32, kind="ExternalInput").ap()
    wgu_d = nc.dram_tensor("wgu", [H, 2 * D], F32, kind="ExternalInput").ap()
    wd_d = nc.dram_tensor("wd", [D, H], F32, kind="ExternalInput").ap()
    y_d = nc.dram_tensor("y", [T, H], F32, kind="ExternalOutput").ap()

    with tile.TileContext(nc) as tc, ExitStack() as ctx:
        const = ctx.enter_context(tc.tile_pool(name="const", bufs=1))
        ident = const.tile([P, P], F32)
        make_identity(nc, ident)

        slab = ctx.enter_context(tc.tile_pool(name="slab", bufs=1))
        xt = slab.tile([P, KH * TH], F32R, tag="xt")
        act = slab.tile([P, KD * TH], F32R, tag="act")

        xstage = ctx.enter_context(tc.tile_pool(name="xstage", bufs=5))
        wstage = ctx.enter_context(tc.tile_pool(name="wstage", bufs=w_bufs))
        wr = ctx.enter_context(tc.tile_pool(name="wr", bufs=w_bufs))
        stmp = ctx.enter_context(tc.tile_pool(name="stmp", bufs=2))
        yout = ctx.enter_context(tc.tile_pool(name="yout", bufs=2))
        ps = ctx.enter_context(tc.tile_pool(name="ps", bufs=8, space="PSUM"))

        for h in range(halves):
            t0 = h * TH
            # ---- transpose X half into xt (PE transpose via identity) ----
            for tt in range(TT):
                for hc in range(NXC):
                    xs = xstage.tile([P, XC], F32, name="xs")
                    nc.sync.dma_start(
                        xs[:],
                        x_d[t0 + tt * P:t0 + (tt + 1) * P, hc * XC:(hc + 1) * XC])
                    for kk in range(XC // P):
                        k = hc * (XC // P) + kk
                        pst = ps.tile([P, P], F32, tag="ps", name="pst")
                        nc.tensor.matmul(pst[:], xs[:, kk * P:(kk + 1) * P],
                                         ident[:], is_transpose=True)
                        nc.vector.tensor_copy(
                            xt[:, k * TH + tt * P:k * TH + (tt + 1) * P], pst[:])

            # ---- MM1 (gate/up) + SwiGLU ----
            for gg in range(GG):
                psg, psu = [], []
                for which, lst in ((0, psg), (1, psu)):
                    col0 = which * D + gg * MG * P
                    for m in range(MG):
                        lst.append(ps.tile([P, TH], F32, tag="ps", name="psgu"))
                    for k in range(KH):
                        wc = wstage.tile([P, MG * P], F32, tag="wc", name="wc")
                        nc.gpsimd.dma_start(
                            wc[:], wgu_d[k * P:(k + 1) * P, col0:col0 + MG * P])
                        wrt = wr.tile([P, MG * P], F32R, tag="wrt", name="wrt")
                        nc.vector.tensor_copy(wrt[:], wc[:])
                        for m in range(MG):
                            nc.tensor.matmul(
                                lst[m][:], wrt[:, m * P:(m + 1) * P],
                                xt[:, k * TH:(k + 1) * TH],
                                start=(k == 0), stop=(k == KH - 1))
                for m in range(MG):
                    st = stmp.tile([P, TH], F32, name="st")
                    nc.scalar.activation(st[:], psg[m][:], SIGMOID)
                    gt = stmp.tile([P, TH], F32, tag="gt", name="gt")
                    nc.vector.tensor_mul(gt[:], psg[m][:], st[:])
                    d_tile = gg * MG + m
                    nc.vector.tensor_mul(
                        act[:, d_tile * TH:(d_tile + 1) * TH], psu[m][:], gt[:])

            # ---- MM2 (down projection) ----
            for nh in range(NH):
                psy = [ps.tile([P, NW], F32, tag="ps", name="psy")
                       for _ in range(TT)]
                for kd in range(KD):
                    wc = wstage.tile([P, NW], F32, tag="wc", name="wc")
                    nc.gpsimd.dma_start(
                        wc[:], wd_d[kd * P:(kd + 1) * P, nh * NW:(nh + 1) * NW])
                    wrt = wr.tile([P, NW], F32R, tag="wrt", name="wrt")
                    nc.vector.tensor_copy(wrt[:], wc[:])
                    for mt in range(TT):
                        nc.tensor.matmul(
                            psy[mt][:],
                            act[:, kd * TH + mt * P:kd * TH + (mt + 1) * P],
                            wrt[:], start=(kd == 0), stop=(kd == KD - 1))
                for mt in range(TT):
                    yo = yout.tile([P, NW], F32, name="yo")
                    nc.scalar.activation(yo[:], psy[mt][:], COPY)
                    nc.sync.dma_start(
                        y_d[t0 + mt * P:t0 + (mt + 1) * P, nh * NW:(nh + 1) * NW],
                        yo[:])

    nc.compile()
    return nc


def get_program():
    global _cached_nc
    if _cached_nc is None:
        _cached_nc = _build_program()
    return _cached_nc


def kernel(hidden_states, gate_up_proj, down_proj, run_index=None, _trace=False):
    hs = np.ascontiguousarray(np.asarray(hidden_states, dtype=np.float32))
    wgu = np.ascontiguousarray(np.asarray(gate_up_proj, dtype=np.float32))
    wd = np.ascontiguousarray(np.asarray(down_proj, dtype=np.float32))
    assert hs.shape == (E * T, H) and wgu.shape == (E, H, 2 * D) \
        and wd.shape == (E, D, H)

    nc = get_program()
    in_maps = [{"x": hs[e * T:(e + 1) * T], "wgu": wgu[e], "wd": wd[e]}
               for e in range(E)]
    res = run_bass_kernel_spmd(nc, in_maps, core_ids=list(range(E)),
                               trace=_trace)
    out = np.empty((E * T, H), dtype=np.float32)
    for e in range(E):
        out[e * T:(e + 1) * T] = res.results[e]["y"]
    if _trace:
        kernel.last_result = res
    return out



# revision 3
# speedup vs baseline: 1.0228x; 1.0228x over previous
"""Llama4 MoE experts (grouped GEMM + SwiGLU) on 8 Trainium2 NeuronCores.

Expert-parallel: core e computes expert e's token block
  Y_e = (silu(X_e @ Wg_e) * (X_e @ Wu_e)) @ Wd_e
with X_e = hidden_states[e*1024:(e+1)*1024]. No collectives needed.

Single token pass in bf16 (weights stream from HBM exactly once):
  - X, Wgu, Wd are cast fp32->bf16 in the DMA itself (SWDGE cast), so no
    on-chip cast copies; matmuls run at full rate (1 col/cycle).
  - xt (transposed X) and act (SwiGLU output) live in SBUF as bf16 slabs
    (64 KiB/partition each) so all 1024 tokens are processed in one pass.
  - Prologue (PE transpose of X) is interleaved with the first MM1 pair:
    token-half-0 matmuls run while token-half-1 is still loading.
  - MM1: out[dcol_tile, tokens] = Wgu_tile.T @ xt, accumulated over H in
    PSUM (4 banks per gate/up pair, two pairs in flight); SwiGLU =
    sigmoid on ScalarE + two DVE muls, written bf16 into act.
  - MM2: out[token_tile, H_cols] = act_tile.T @ Wd_chunk, accumulated
    over D in PSUM (8 banks), evicted on DVE, DMA out on HWDGE.
Measured: 1.435 ms HW (slowest of 8 cores), rel err 4.1e-3; PE busy 94%
at ~220 ns per [128x128x512] matmul (streaming floor 213 ns).
"""
from contextlib import ExitStack

import numpy as np

import concourse.bass as bass
import concourse.tile as tile
from concourse import bacc, mybir
from concourse.bass_utils import run_bass_kernel_spmd
from concourse.masks import make_identity

P = 128
F32 = mybir.dt.float32
BF16 = mybir.dt.bfloat16
SIGMOID = mybir.ActivationFunctionType.Sigmoid
COPY = mybir.ActivationFunctionType.Copy

E = 8            # experts == cores
T = 1024         # tokens per expert
H = 4096         # hidden
D = 4096         # expert (intermediate) dim

_cached_nc = None


def _build_program(T=T, H=H, D=D, w_bufs=7, xs_bufs=8, st_bufs=3,
                   y_bufs=3, vec_evict=True, hoist_wd=True,
                   phases=(0, 1, 2)):
    KH = H // P          # 32 contraction tiles for MM1
    KD = D // P          # 32 contraction tiles for MM2

    nc = bacc.Bacc("TRN2", target_bir_lowering=False, debug=False)
    x_d = nc.dram_tensor("x", [T, H], F32, kind="ExternalInput").ap()
    wgu_d = nc.dram_tensor("wgu", [H, 2 * D], F32, kind="ExternalInput").ap()
    wd_d = nc.dram_tensor("wd", [D, H], F32, kind="ExternalInput").ap()
    y_d = nc.dram_tensor("y", [T, H], F32, kind="ExternalOutput").ap()

    with tile.TileContext(nc) as tc, ExitStack() as ctx:
        const = ctx.enter_context(tc.tile_pool(name="const", bufs=1))
        ident = const.tile([P, P], BF16)
        make_identity(nc, ident)

        slab = ctx.enter_context(tc.tile_pool(name="slab", bufs=1))
        xt = slab.tile([P, KH * T], BF16, tag="xt")    # [h%128, k*T + t]
        act = slab.tile([P, KD * T], BF16, tag="act")  # [d%128, kd*T + t]

        xsp = ctx.enter_context(tc.tile_pool(name="xsp", bufs=xs_bufs))
        wst = ctx.enter_context(tc.tile_pool(name="wst", bufs=w_bufs))
        stm = ctx.enter_context(tc.tile_pool(name="stm", bufs=st_bufs))
        yout = ctx.enter_context(tc.tile_pool(name="yout", bufs=y_bufs))
        ps = ctx.enter_context(tc.tile_pool(name="ps", bufs=8, space="PSUM"))

        # ---- Phase 0: transpose X into xt (PE transpose, bf16) ----
        # Token-tile groups of 4 so each PSUM bank holds 512 contiguous
        # tokens of one h-block -> contiguous DVE copies into xt.
        def emit_tg(tg):
            for hc in range(2):
                xs4 = []
                for j in range(4):
                    tt = tg * 4 + j
                    xs = xsp.tile([P, 2048], BF16, tag="xs", name="xs")
                    nc.gpsimd.dma_start(
                        xs, x_d[tt * P:(tt + 1) * P, hc * 2048:(hc + 1) * 2048])
                    xs4.append(xs)
                for c in range(16):
                    k = hc * 16 + c
                    pst = ps.tile([P, 512], BF16, tag="ps", name="pst")
                    for j in range(4):
                        nc.tensor.matmul(pst[:, j * P:(j + 1) * P],
                                         xs4[j][:, c * P:(c + 1) * P],
                                         ident[:], is_transpose=True)
                    nc.vector.tensor_copy(
                        xt[:, k * T + tg * 512:k * T + (tg + 1) * 512], pst[:])

        if 0 in phases and 1 not in phases:
            emit_tg(0)
            emit_tg(1)

        def dma_wd(nh, ks):
            wdt = wst.tile([P, 2048], BF16, tag="w", name="wdt")
            nc.gpsimd.dma_start(
                wdt[:].rearrange("p (k c) -> p k c", k=4),
                wd_d[ks * 512:(ks + 1) * 512, nh * 512:(nh + 1) * 512]
                .rearrange("(k p) c -> p k c", p=P))
            return wdt

        # First wd chunk DMA'd up front (its slot is held through MM1) so
        # the PE never waits on wd at the MM1 -> MM2 phase boundary.
        wdt0 = dma_wd(0, 0) if (hoist_wd and 2 in phases) else None

        # ---- Phase 1: MM1 (gate/up) + SwiGLU ----
        # Per d-tile pair mp: gate cols [mp*128, +128), up cols [D+mp*128).
        # Accumulate over all H into 4 PSUM banks (2 token halves x g/u).
        def dma_wgu(mp, ks, which):
            col = which * D + mp * P
            wt = wst.tile([P, 1024], BF16, tag="w", name="wgu_t")
            nc.gpsimd.dma_start(
                wt[:].rearrange("p (k c) -> p k c", k=8),
                wgu_d[ks * 1024:(ks + 1) * 1024, col:col + P]
                .rearrange("(k p) c -> p k c", p=P))
            return wt

        def swiglu(mp, hh, psg_t, psu_t):
            st = stm.tile([P, 512], F32, tag="st", name="st")
            nc.scalar.activation(st[:], psg_t[:], SIGMOID)
            gt = stm.tile([P, 512], F32, tag="gt", name="gt")
            nc.vector.tensor_mul(gt[:], psg_t[:], st[:])
            nc.vector.tensor_mul(
                act[:, mp * T + hh * 512:mp * T + (hh + 1) * 512],
                psu_t[:], gt[:])

        def emit_pair_half(mp, hh):
            # One token half of pair mp, with its own weight DMAs.
            psg_t = ps.tile([P, 512], F32, tag="ps", name="psg")
            psu_t = ps.tile([P, 512], F32, tag="ps", name="psu")
            for ks in range(4):
                wgt = dma_wgu(mp, ks, 0)
                wut = dma_wgu(mp, ks, 1)
                for kk in range(8):
                    k = ks * 8 + kk
                    rhs = xt[:, k * T + hh * 512:k * T + (hh + 1) * 512]
                    nc.tensor.matmul(psg_t[:], wgt[:, kk * P:(kk + 1) * P],
                                     rhs, start=(k == 0), stop=(k == KH - 1))
                    nc.tensor.matmul(psu_t[:], wut[:, kk * P:(kk + 1) * P],
                                     rhs, start=(k == 0), stop=(k == KH - 1))
            swiglu(mp, hh, psg_t, psu_t)

        def emit_pair(mp):
            psg = [ps.tile([P, 512], F32, tag="ps", name="psg")
                   for _ in range(2)]
            psu = [ps.tile([P, 512], F32, tag="ps", name="psu")
                   for _ in range(2)]
            for ks in range(4):
                wgt = dma_wgu(mp, ks, 0)
                wut = dma_wgu(mp, ks, 1)
                for kk in range(8):
                    k = ks * 8 + kk
                    first = (k == 0)
                    last = (k == KH - 1)
                    for hh in range(2):
                        nc.tensor.matmul(
                            psg[hh][:], wgt[:, kk * P:(kk + 1) * P],
                            xt[:, k * T + hh * 512:k * T + (hh + 1) * 512],
                            start=first, stop=last)
                    for hh in range(2):
                        nc.tensor.matmul(
                            psu[hh][:], wut[:, kk * P:(kk + 1) * P],
                            xt[:, k * T + hh * 512:k * T + (hh + 1) * 512],
                            start=first, stop=last)
            for hh in range(2):
                swiglu(mp, hh, psg[hh], psu[hh])

        if 0 in phases and 1 in phases:
            # Interleave: pair 0's token-half-0 matmuls run while token
            # half 1 is still being loaded/transposed. Pair 0 re-streams
            # its weights for half 1 (4 MB extra of 192 MB total).
            emit_tg(0)
            emit_pair_half(0, 0)
            emit_tg(1)
            emit_pair_half(0, 1)
            for mp in range(1, 32):
                emit_pair(mp)
        elif 1 in phases:
            for mp in range(32):
                emit_pair(mp)

        # ---- Phase 2: MM2 (down projection) ----
        for nh in range(8) if 2 in phases else ():
            psy = [ps.tile([P, 512], F32, tag="ps", name="psy")
                   for _ in range(8)]
            for ks in range(8):
                if nh == 0 and ks == 0 and wdt0 is not None:
                    wdt = wdt0
                else:
                    wdt = dma_wd(nh, ks)
                for kk in range(4):
                    kd = ks * 4 + kk
                    for mt in range(8):
                        nc.tensor.matmul(
                            psy[mt][:],
                            act[:, kd * T + mt * P:kd * T + (mt + 1) * P],
                            wdt[:, kk * 512:(kk + 1) * 512],
                            start=(kd == 0), stop=(kd == KD - 1))
            for mt in range(8):
                yo = yout.tile([P, 512], F32, name="yo")
                if vec_evict:
                    nc.vector.tensor_copy(yo[:], psy[mt][:])
                else:
                    nc.scalar.activation(yo[:], psy[mt][:], COPY)
                nc.sync.dma_start(
                    y_d[mt * P:(mt + 1) * P, nh * 512:(nh + 1) * 512], yo[:])

    nc.compile()
    return nc


def get_program():
    global _cached_nc
    if _cached_nc is None:
        _cached_nc = _build_program()
    return _cached_nc


def kernel(hidden_states, gate_up_proj, down_proj, run_index=None, _trace=False):
    hs = np.ascontiguousarray(np.asarray(hidden_states, dtype=np.float32))
    wgu = np.ascontiguousarray(np.asarray(gate_up_proj, dtype=np.float32))
    wd = np.ascontiguousarray(np.asarray(down_proj, dtype=np.float32))
    assert hs.shape == (E * T, H) and wgu.shape == (E, H, 2 * D) \
        and wd.shape == (E, D, H)

    nc = get_program()
    in_maps = [{"x": hs[e * T:(e + 1) * T], "wgu": wgu[e], "wd": wd[e]}
               for e in range(E)]
    res = run_bass_kernel_spmd(nc, in_maps, core_ids=list(range(E)),
                               trace=_trace)
    out = np.empty((E * T, H), dtype=np.float32)
    for e in range(E):
        out[e * T:(e + 1) * T] = res.results[e]["y"]
    if _trace:
        kernel.last_result = res
    return out


# revision 5
# speedup vs baseline: 1.0249x; 1.0020x over previous
"""Llama4 MoE experts (grouped GEMM + SwiGLU) on 8 Trainium2 NeuronCores.

Expert-parallel: core e computes expert e's token block
  Y_e = (silu(X_e @ Wg_e) * (X_e @ Wu_e)) @ Wd_e
with X_e = hidden_states[e*1024:(e+1)*1024]. No collectives needed.

Single token pass in bf16 (weights stream from HBM exactly once):
  - X, Wgu, Wd are cast fp32->bf16 in the DMA itself (SWDGE cast), so no
    on-chip cast copies; matmuls run at full rate (1 col/cycle).
  - xt (transposed X) and act (SwiGLU output) live in SBUF as bf16 slabs
    (64 KiB/partition each) so all 1024 tokens go through in one pass.
  - Prologue (PE transpose of X) is interleaved with the first MM1 pair:
    token-half-0 matmuls run while token-half-1 is still loading.
  - MM1: out[dcol_tile, tokens] = Wgu_tile.T @ xt, accumulated over H in
    PSUM (4 banks per gate/up pair, two pairs in flight); SwiGLU =
    sigmoid on ScalarE + two DVE muls, written bf16 into act.
  - MM2: out[token_tile, H_cols] = act_tile.T @ Wd_chunk, accumulated
    over D in PSUM (8 banks); evictions alternate DVE/ACT and the y DMAs
    alternate both HWDGE rings so PSUM-bank release never throttles PE.
Measured: 1.418 ms HW (slowest of 8 cores; mean 1.408 ms), rel err
4.1e-3; PE 96%+ busy at ~220 ns per [128x128x512] matmul (floor 213).
"""
from contextlib import ExitStack

import numpy as np

import concourse.bass as bass
import concourse.tile as tile
from concourse import bacc, mybir
from concourse.bass_utils import run_bass_kernel_spmd
from concourse.masks import make_identity

P = 128
F32 = mybir.dt.float32
BF16 = mybir.dt.bfloat16
SIGMOID = mybir.ActivationFunctionType.Sigmoid
COPY = mybir.ActivationFunctionType.Copy

E = 8            # experts == cores
T = 1024         # tokens per expert
H = 4096         # hidden
D = 4096         # expert (intermediate) dim

_cached_nc = None


def _build_program(T=T, H=H, D=D, w_bufs=7, xs_bufs=8, st_bufs=2,
                   y_bufs=5, vec_evict=True, hoist_wd=True,
                   phases=(0, 1, 2)):
    KH = H // P          # 32 contraction tiles for MM1
    KD = D // P          # 32 contraction tiles for MM2

    nc = bacc.Bacc("TRN2", target_bir_lowering=False, debug=False)
    x_d = nc.dram_tensor("x", [T, H], F32, kind="ExternalInput").ap()
    wgu_d = nc.dram_tensor("wgu", [H, 2 * D], F32, kind="ExternalInput").ap()
    wd_d = nc.dram_tensor("wd", [D, H], F32, kind="ExternalInput").ap()
    y_d = nc.dram_tensor("y", [T, H], F32, kind="ExternalOutput").ap()

    with tile.TileContext(nc) as tc, ExitStack() as ctx:
        const = ctx.enter_context(tc.tile_pool(name="const", bufs=1))
        ident = const.tile([P, P], BF16)
        make_identity(nc, ident)

        slab = ctx.enter_context(tc.tile_pool(name="slab", bufs=1))
        xt = slab.tile([P, KH * T], BF16, tag="xt")    # [h%128, k*T + t]
        act = slab.tile([P, KD * T], BF16, tag="act")  # [d%128, kd*T + t]

        xsp = ctx.enter_context(tc.tile_pool(name="xsp", bufs=xs_bufs))
        wst = ctx.enter_context(tc.tile_pool(name="wst", bufs=w_bufs))
        stm = ctx.enter_context(tc.tile_pool(name="stm", bufs=st_bufs))
        yout = ctx.enter_context(tc.tile_pool(name="yout", bufs=y_bufs))
        ps = ctx.enter_context(tc.tile_pool(name="ps", bufs=8, space="PSUM"))

        # ---- Phase 0: transpose X into xt (PE transpose, bf16) ----
        # Token-tile groups of 4 so each PSUM bank holds 512 contiguous
        # tokens of one h-block -> contiguous DVE copies into xt.
        def emit_tg(tg):
            for hc in range(2):
                xs4 = []
                for j in range(4):
                    tt = tg * 4 + j
                    xs = xsp.tile([P, 2048], BF16, tag="xs", name="xs")
                    nc.gpsimd.dma_start(
                        xs, x_d[tt * P:(tt + 1) * P, hc * 2048:(hc + 1) * 2048])
                    xs4.append(xs)
                for c in range(16):
                    k = hc * 16 + c
                    pst = ps.tile([P, 512], BF16, tag="ps", name="pst")
                    for j in range(4):
                        nc.tensor.matmul(pst[:, j * P:(j + 1) * P],
                                         xs4[j][:, c * P:(c + 1) * P],
                                         ident[:], is_transpose=True)
                    nc.vector.tensor_copy(
                        xt[:, k * T + tg * 512:k * T + (tg + 1) * 512], pst[:])

        if 0 in phases and 1 not in phases:
            emit_tg(0)
            emit_tg(1)

        def dma_wd(nh, ks):
            wdt = wst.tile([P, 2048], BF16, tag="w", name="wdt")
            nc.gpsimd.dma_start(
                wdt[:].rearrange("p (k c) -> p k c", k=4),
                wd_d[ks * 512:(ks + 1) * 512, nh * 512:(nh + 1) * 512]
                .rearrange("(k p) c -> p k c", p=P))
            return wdt

        # First wd chunk DMA'd up front (its slot is held through MM1) so
        # the PE never waits on wd at the MM1 -> MM2 phase boundary.
        wdt0 = dma_wd(0, 0) if (hoist_wd and 2 in phases) else None

        # ---- Phase 1: MM1 (gate/up) + SwiGLU ----
        # Per d-tile pair mp: gate cols [mp*128, +128), up cols [D+mp*128).
        # Accumulate over all H into 4 PSUM banks (2 token halves x g/u).
        def dma_wgu(mp, ks, which):
            col = which * D + mp * P
            wt = wst.tile([P, 1024], BF16, tag="w", name="wgu_t")
            nc.gpsimd.dma_start(
                wt[:].rearrange("p (k c) -> p k c", k=8),
                wgu_d[ks * 1024:(ks + 1) * 1024, col:col + P]
                .rearrange("(k p) c -> p k c", p=P))
            return wt

        def swiglu(mp, hh, psg_t, psu_t):
            st = stm.tile([P, 512], F32, tag="st", name="st")
            nc.scalar.activation(st[:], psg_t[:], SIGMOID)
            gt = stm.tile([P, 512], F32, tag="gt", name="gt")
            nc.vector.tensor_mul(gt[:], psg_t[:], st[:])
            nc.vector.tensor_mul(
                act[:, mp * T + hh * 512:mp * T + (hh + 1) * 512],
                psu_t[:], gt[:])

        def emit_pair_half(mp, hh):
            # One token half of pair mp, with its own weight DMAs.
            psg_t = ps.tile([P, 512], F32, tag="ps", name="psg")
            psu_t = ps.tile([P, 512], F32, tag="ps", name="psu")
            for ks in range(4):
                wgt = dma_wgu(mp, ks, 0)
                wut = dma_wgu(mp, ks, 1)
                for kk in range(8):
                    k = ks * 8 + kk
                    rhs = xt[:, k * T + hh * 512:k * T + (hh + 1) * 512]
                    nc.tensor.matmul(psg_t[:], wgt[:, kk * P:(kk + 1) * P],
                                     rhs, start=(k == 0), stop=(k == KH - 1))
                    nc.tensor.matmul(psu_t[:], wut[:, kk * P:(kk + 1) * P],
                                     rhs, start=(k == 0), stop=(k == KH - 1))
            swiglu(mp, hh, psg_t, psu_t)

        def emit_pair(mp):
            psg = [ps.tile([P, 512], F32, tag="ps", name="psg")
                   for _ in range(2)]
            psu = [ps.tile([P, 512], F32, tag="ps", name="psu")
                   for _ in range(2)]
            for ks in range(4):
                wgt = dma_wgu(mp, ks, 0)
                wut = dma_wgu(mp, ks, 1)
                for kk in range(8):
                    k = ks * 8 + kk
                    first = (k == 0)
                    last = (k == KH - 1)
                    for hh in range(2):
                        nc.tensor.matmul(
                            psg[hh][:], wgt[:, kk * P:(kk + 1) * P],
                            xt[:, k * T + hh * 512:k * T + (hh + 1) * 512],
                            start=first, stop=last)
                    for hh in range(2):
                        nc.tensor.matmul(
                            psu[hh][:], wut[:, kk * P:(kk + 1) * P],
                            xt[:, k * T + hh * 512:k * T + (hh + 1) * 512],
                            start=first, stop=last)
            for hh in range(2):
                swiglu(mp, hh, psg[hh], psu[hh])

        if 0 in phases and 1 in phases:
            # Interleave: pair 0's token-half-0 matmuls run while token
            # half 1 is still being loaded/transposed. Pair 0 re-streams
            # its weights for half 1 (4 MB extra of 192 MB total).
            emit_tg(0)
            emit_pair_half(0, 0)
            emit_tg(1)
            emit_pair_half(0, 1)
            for mp in range(1, 32):
                emit_pair(mp)
        elif 1 in phases:
            for mp in range(32):
                emit_pair(mp)

        # ---- Phase 2: MM2 (down projection) ----
        for nh in range(8) if 2 in phases else ():
            psy = [ps.tile([P, 512], F32, tag="ps", name="psy")
                   for _ in range(8)]
            for ks in range(8):
                if nh == 0 and ks == 0 and wdt0 is not None:
                    wdt = wdt0
                else:
                    wdt = dma_wd(nh, ks)
                for kk in range(4):
                    kd = ks * 4 + kk
                    for mt in range(8):
                        nc.tensor.matmul(
                            psy[mt][:],
                            act[:, kd * T + mt * P:kd * T + (mt + 1) * P],
                            wdt[:, kk * 512:(kk + 1) * 512],
                            start=(kd == 0), stop=(kd == KD - 1))
            for mt in range(8):
                yo = yout.tile([P, 512], F32, name="yo")
                # Alternate copy engines (DVE/ACT) and HWDGE rings
                # (SP/ACT) so PSUM-bank release at nh boundaries isn't
                # throttled by one engine or one DMA queue.
                if vec_evict and mt % 2 == 0:
                    nc.vector.tensor_copy(yo[:], psy[mt][:])
                else:
                    nc.scalar.activation(yo[:], psy[mt][:], COPY)
                dma_eng = nc.sync if mt % 2 == 0 else nc.scalar
                dma_eng.dma_start(
                    y_d[mt * P:(mt + 1) * P, nh * 512:(nh + 1) * 512], yo[:])

    nc.compile()
    return nc


def get_program():
    global _cached_nc
    if _cached_nc is None:
        _cached_nc = _build_program()
    return _cached_nc


def kernel(hidden_states, gate_up_proj, down_proj, run_index=None, _trace=False):
    hs = np.ascontiguousarray(np.asarray(hidden_states, dtype=np.float32))
    wgu = np.ascontiguousarray(np.asarray(gate_up_proj, dtype=np.float32))
    wd = np.ascontiguousarray(np.asarray(down_proj, dtype=np.float32))
    assert hs.shape == (E * T, H) and wgu.shape == (E, H, 2 * D) \
        and wd.shape == (E, D, H)

    nc = get_program()
    in_maps = [{"x": hs[e * T:(e + 1) * T], "wgu": wgu[e], "wd": wd[e]}
               for e in range(E)]
    res = run_bass_kernel_spmd(nc, in_maps, core_ids=list(range(E)),
                               trace=_trace)
    out = np.empty((E * T, H), dtype=np.float32)
    for e in range(E):
        out[e * T:(e + 1) * T] = res.results[e]["y"]
    if _trace:
        kernel.last_result = res
    return out
